# revision 5
# baseline (speedup 1.0000x reference)
"""Trainium2 Bass kernel for nn_BRNN_8151847927833.

Binary RNN: B=64 seqs, T=512 steps, d_model=1024, 6 binary FF layers per
step, then a small head + log_softmax + NLL loss averaged over (t, b).

Strategy (data-parallel over batch, 8 cores x 8 sequences):
  - All weights are +-1 (sign of latents), thresholds are small integers.
    Matmuls are therefore EXACT in low precision: products are +-1 and
    PSUM accumulates in fp32.
  - Activations are kept transposed: x^T stored as [128 partitions, 64]
    where column = m_chunk*8 + b (8 chunks of 128 dims x 8 batch).
    Weight-stationary matmuls (lhsT = W chunk [128k x 128m], moving
    rhs = x^T chunk [128, 8]) produce the NEXT transposed layout
    directly -> zero transposes in the whole recurrence.
  - sign((pre - thr)/sqrt(d)) == Sign(pre - (thr - 0.5)) exactly, since
    pre and thr are integers (ties at 0 go to +1).
  - The head + log-softmax + token-gather do NOT feed the recurrence, so
    they are deferred: the 128 "read" dims per step are stored to a
    [128, T*8] buffer and processed as 32 dense batched matmul tiles
    after the T-loop.  No max-subtraction needed: |logits| <= 8.
  - Each core returns per-partition partial sums of (logsumexp - logit_tok);
    the host sums across cores and divides by B*T.
"""

import math
import sys

import numpy as np

sys.path.insert(0, "/opt/trn_rl_repo")

import ml_dtypes  # noqa: E402

import concourse.bass as bass  # noqa: E402
import concourse.bacc as bacc  # noqa: E402
import concourse.mybir as mybir  # noqa: E402
from concourse.tile import TileContext  # noqa: E402
from concourse.bass_utils import run_bass_kernel_spmd  # noqa: E402

F32 = mybir.dt.float32
BF16 = mybir.dt.bfloat16
FP8 = mybir.dt.float8e4
NP_BF16 = ml_dtypes.bfloat16
NP_FP8 = ml_dtypes.float8_e4m3

D = 1024          # d_model
KC = 8            # contraction chunks of 128
MC = 8            # output chunks of 128
NUMFF = 6
VOCAB = 128
READ = 128
CARRY = 896
BL = 8            # batch per core
NCORES = 8
LOGIT_SCALE = 1.0 / 16.0

# weight dtype for the FF stack / head (fp8 -> fast weight load)
W_DT = FP8
W_NP = NP_FP8


def build_nc(T):
    """Build the SPMD Bass kernel for T timesteps (BL sequences/core)."""
    ntile = T * BL // 128  # tail tiles over (t, b)
    assert T * BL % 128 == 0

    nc = bacc.Bacc("TRN2", target_bir_lowering=False)
    wff = nc.dram_tensor("wff", [128, NUMFF * KC * D], W_DT, kind="ExternalInput")
    emb = nc.dram_tensor("emb", [128, T * BL], BF16, kind="ExternalInput")
    x0 = nc.dram_tensor("x0", [128, MC * BL], BF16, kind="ExternalInput")
    thr = nc.dram_tensor("thr", [128, NUMFF * MC * BL], F32, kind="ExternalInput")
    headw = nc.dram_tensor("headw", [128, VOCAB], W_DT, kind="ExternalInput")
    oneh = nc.dram_tensor("oneh", [128, ntile * VOCAB], F32, kind="ExternalInput")
    res = nc.dram_tensor("res", [128, 1], F32, kind="ExternalOutput")

    AT = mybir.ActivationFunctionType
    ALU = mybir.AluOpType

    with TileContext(nc) as tc:
        with (
            tc.tile_pool(name="const", bufs=1) as cpool,
            tc.tile_pool(name="work", bufs=4) as wpool,
            tc.tile_pool(name="hpool", bufs=3) as hpool,
            tc.tile_pool(name="pspool", bufs=4, space="PSUM") as pspool,
            tc.tile_pool(name="ps2pool", bufs=2, space="PSUM") as ps2pool,
        ):
            # ---- resident inputs ----
            wsb = cpool.tile([128, NUMFF * KC * D], W_DT, tag="wsb")
            for i in range(NUMFF):
                nc.sync.dma_start(
                    out=wsb[:, i * KC * D : (i + 1) * KC * D],
                    in_=wff[:, i * KC * D : (i + 1) * KC * D],
                )
            embsb = cpool.tile([128, T * BL], BF16, tag="embsb")
            nc.sync.dma_start(out=embsb[:, :], in_=emb[:, :])
            thrsb = cpool.tile([128, NUMFF * MC * BL], F32, tag="thrsb")
            nc.sync.dma_start(out=thrsb[:, :], in_=thr[:, :])
            headsb = cpool.tile([128, VOCAB], W_DT, tag="headsb")
            nc.sync.dma_start(out=headsb[:, :], in_=headw[:, :])
            onehsb = cpool.tile([128, ntile * VOCAB], F32, tag="onehsb")
            nc.sync.dma_start(out=onehsb[:, :], in_=oneh[:, :])
            xb = cpool.tile([128, MC * BL], BF16, tag="xb")
            nc.sync.dma_start(out=xb[:, :], in_=x0[:, :])

            # ---- persistent state ----
            readst = cpool.tile([128, T * BL], BF16, tag="readst")
            sumexp = cpool.tile([128, ntile], F32, tag="sumexp")
            tokl = cpool.tile([128, ntile], F32, tag="tokl")

            # ---- recurrence ----
            for t in range(T):
                src = xb
                for i in range(NUMFF):
                    ps = pspool.tile([128, MC * BL], F32, tag="ps")
                    for m in range(MC):
                        lo = (i * KC) * D + m * 128
                        for k in range(KC):
                            nc.tensor.matmul(
                                ps[:, m * BL : (m + 1) * BL],
                                wsb[:, k * D + lo : k * D + lo + 128],
                                src[:, k * BL : (k + 1) * BL],
                                start=(k == 0),
                                stop=(k == KC - 1),
                            )
                    tmp = wpool.tile([128, MC * BL], BF16, tag="tmp")
                    # tmp = pre - (thr - 0.5)  (sign-safe integer compare)
                    nc.vector.scalar_tensor_tensor(
                        out=tmp[:, :],
                        in0=ps[:, :],
                        scalar=0.0,
                        in1=thrsb[:, i * MC * BL : (i + 1) * MC * BL],
                        op0=ALU.add,
                        op1=ALU.subtract,
                    )
                    if i < NUMFF - 1:
                        h = hpool.tile([128, MC * BL], BF16, tag="h")
                        nc.scalar.sign(h[:, :], tmp[:, :])
                        src = h
                    else:
                        # carry chunks 0..6 -> next x; read chunk 7 -> store
                        nc.scalar.sign(xb[:, 0 : CARRY // 16], tmp[:, 0 : CARRY // 16])
                        nc.scalar.sign(
                            readst[:, t * BL : (t + 1) * BL],
                            tmp[:, CARRY // 16 : MC * BL],
                        )
                        nc.vector.tensor_copy(
                            xb[:, CARRY // 16 : MC * BL],
                            embsb[:, t * BL : (t + 1) * BL],
                        )

            # ---- deferred head + loss ----
            for j in range(ntile):
                ps2 = ps2pool.tile([128, VOCAB], F32, tag="ps2")
                nc.tensor.matmul(
                    ps2[:, :],
                    readst[:, j * 128 : (j + 1) * 128],
                    headsb[:, :],
                    start=True,
                    stop=True,
                )
                expt = wpool.tile([128, VOCAB], F32, tag="expt")
                nc.scalar.activation(
                    expt[:, :],
                    ps2[:, :],
                    AT.Exp,
                    scale=LOGIT_SCALE,
                    accum_out=sumexp[:, j : j + 1],
                )
                junk = wpool.tile([128, VOCAB], F32, tag="junk")
                nc.vector.scalar_tensor_tensor(
                    out=junk[:, :],
                    in0=ps2[:, :],
                    scalar=LOGIT_SCALE,
                    in1=onehsb[:, j * VOCAB : (j + 1) * VOCAB],
                    op0=ALU.mult,
                    op1=ALU.mult,
                    accum_out=tokl[:, j : j + 1],
                )

            lse = cpool.tile([128, ntile], F32, tag="lse")
            nc.scalar.activation(lse[:, :], sumexp[:, :], AT.Ln)
            r1 = cpool.tile([128, 1], F32, tag="r1")
            r2 = cpool.tile([128, 1], F32, tag="r2")
            po = cpool.tile([128, 1], F32, tag="po")
            nc.vector.tensor_reduce(
                r1[:, :], lse[:, :], axis=mybir.AxisListType.X, op=ALU.add
            )
            nc.vector.tensor_reduce(
                r2[:, :], tokl[:, :], axis=mybir.AxisListType.X, op=ALU.add
            )
            nc.vector.scalar_tensor_tensor(
                out=po[:, :],
                in0=r1[:, :],
                scalar=0.0,
                in1=r2[:, :],
                op0=ALU.add,
                op1=ALU.subtract,
            )
            nc.sync.dma_start(out=res[:, :], in_=po[:, :])

    return nc


def _sgn(a):
    return np.where(np.asarray(a, np.float32) >= 0, 1.0, -1.0).astype(np.float32)


def prepare_inputs(tokens, initial_lat, embed_lat, ff_lat, head_lat, ff_thresh_lat):
    """Host-side transform of the full problem inputs to per-core DRAM maps."""
    tokens = np.asarray(tokens).astype(np.int64)
    B, T = tokens.shape
    E = _sgn(embed_lat)                      # [V, 128]
    W = _sgn(ff_lat)                         # [6, 1024, 1024]
    H = _sgn(head_lat)                       # [128, V]
    x0v = _sgn(initial_lat)                  # [1024]
    R = np.round(np.asarray(ff_thresh_lat, np.float64)).astype(np.float32)  # [6,1024]

    # weights: wff[p, (i*KC+k)*D + mcol] = W[i, k*128+p, mcol]
    wff = (
        W.reshape(NUMFF, KC, 128, D).transpose(2, 0, 1, 3).reshape(128, NUMFF * KC * D)
    ).astype(W_NP)
    headw = H.astype(W_NP)                   # [128 r, V]

    # x0: col = chunk*8 + b, value x0v[chunk*128+p] replicated over b
    x0t = x0v.reshape(MC, 128).T             # [p, chunk]
    x0arr = np.repeat(x0t[:, :, None], BL, axis=2).reshape(128, MC * BL).astype(NP_BF16)

    # thr - 0.5 expanded: col = i*64 + m*8 + b
    thr = (
        R.reshape(NUMFF, MC, 128).transpose(2, 0, 1)[:, :, :, None]
        - 0.5
    )
    thr = np.broadcast_to(thr, (128, NUMFF, MC, BL)).reshape(128, NUMFF * MC * BL)
    thr = np.ascontiguousarray(thr, np.float32)

    ntile = T * BL // 128
    in_maps = []
    for c in range(NCORES):
        tc_ = tokens[c * BL : (c + 1) * BL]  # [8, T]
        # emb[p, t*8+b] = E[tok[b,t], p]
        embc = E[tc_].transpose(2, 1, 0).reshape(128, T * BL).astype(NP_BF16)
        # one-hot over vocab per (t,b) sample, tiled [128 samples x 128 vocab]
        flat = tc_.T.reshape(-1)             # tb = t*8+b
        onehc = (flat[:, None] == np.arange(VOCAB)[None, :]).astype(np.float32)
        onehc = (
            onehc.reshape(ntile, 128, VOCAB).transpose(1, 0, 2).reshape(128, ntile * VOCAB)
        )
        onehc = np.ascontiguousarray(onehc)
        in_maps.append(
            {
                "wff": wff,
                "emb": embc,
                "x0": x0arr,
                "thr": thr,
                "headw": headw,
                "oneh": onehc,
            }
        )
    return in_maps, B, T


def _install_axon_trace_hook():
    """The image's antenv lacks axon_hooks; recreate the NTFF profile hook
    via ctypes against libaxon_pjrt.so (mirrors trn_agent_boot.trn_boot)."""
    import contextlib
    import ctypes
    import types

    try:
        from antenv.axon_hooks import get_axon_ntff_profile_hook  # noqa: F401

        return
    except ImportError:
        pass
    so_path = "/opt/axon/libaxon_pjrt.so"
    lib = ctypes.CDLL(so_path)
    if not hasattr(lib, "axon_start_nrt_profile"):
        return
    lib.axon_start_nrt_profile.argtypes = [
        ctypes.POINTER(ctypes.c_int64),
        ctypes.c_size_t,
    ]
    lib.axon_start_nrt_profile.restype = ctypes.c_int64
    lib.axon_stop_nrt_profile.argtypes = [ctypes.c_char_p]
    lib.axon_stop_nrt_profile.restype = ctypes.c_int64

    @contextlib.contextmanager
    def _hook(output_dir, device_ids):
        import jax

        jax.devices()
        if device_ids:
            ids = (ctypes.c_int64 * len(device_ids))(*device_ids)
            rc = lib.axon_start_nrt_profile(ids, len(device_ids))
        else:
            rc = lib.axon_start_nrt_profile(None, 0)
        if rc != 0:
            raise RuntimeError(f"axon_start_nrt_profile rc={rc}")
        try:
            yield
        finally:
            n = lib.axon_stop_nrt_profile(str(output_dir).encode())
            print(f"profile: {n} file(s) written to {output_dir}", file=sys.stderr)

    import antenv

    mod = types.ModuleType("antenv.axon_hooks")
    mod.get_axon_ntff_profile_hook = lambda: _hook
    sys.modules["antenv.axon_hooks"] = mod
    antenv.axon_hooks = mod

    from concourse import bass_utils as bu

    bu.upload_artifacts = lambda tmpdir: f"local://{tmpdir}"


def run(trace=False, tmpdir=None, **inputs):
    in_maps, B, T = prepare_inputs(**inputs)
    nc = build_nc(T)
    if not nc.is_finalized():
        nc.finalize()
    if trace:
        _install_axon_trace_hook()
    out = run_bass_kernel_spmd(
        nc, in_maps, core_ids=list(range(NCORES)), trace=trace, tmpdir=tmpdir
    )
    total = 0.0
    for r in out.results:
        total += np.asarray(r["res"], np.float64).sum()
    loss = np.float32(total / (B * T))
    return np.asarray(loss, dtype=np.float32), out


def kernel(**inputs):
    loss, _ = run(trace=False, **inputs)
    return loss


if __name__ == "__main__":
    # tiny smoke test
    import jax

    sys.path.insert(0, "/root/problem")
    import reference

    inputs = reference.setup_inputs()
    inputs = {k: np.asarray(v) for k, v in inputs.items()}
    Tsmall = int(sys.argv[1]) if len(sys.argv) > 1 else 16
    inputs["tokens"] = inputs["tokens"][:, :Tsmall]
    expected = np.asarray(reference.reference(**{k: v for k, v in inputs.items()}))
    got = kernel(**inputs)
    rel = abs(float(got) - float(expected)) / max(1e-12, abs(float(expected)))
    print(f"T={Tsmall} expected={expected} got={got} rel_err={rel:.3e}")


# revision 15
# speedup vs baseline: 1.4057x; 1.4057x over previous
"""Trainium2 Bass kernel for nn_BRNN_8151847927833.

Binary RNN: B=64 seqs, T=512 steps, d_model=1024, 6 binary FF layers per
step, then a small head + log_softmax + NLL loss averaged over (t, b).

Strategy (data-parallel over batch, 8 cores x 8 sequences):
  - All weights are +-1 (sign of latents), thresholds are small integers.
    Matmuls are therefore EXACT in low precision: products are +-1 and
    PSUM accumulates in fp32.
  - Activations are kept transposed: x^T stored as [128 partitions, 64]
    where column = m_chunk*8 + b (8 chunks of 128 dims x 8 batch).
    Weight-stationary matmuls (lhsT = W chunk [128k x 128m], moving
    rhs = x^T chunk [128, 8]) produce the NEXT transposed layout
    directly -> zero transposes in the whole recurrence.
  - Activations use a {0,1} encoding (h' = (h+1)/2) so the per-layer
    nonlinearity is a single DVE is_ge against a host-folded threshold
    (thr + colsum(W))/2 — exact integer-vs-half-integer compare, and no
    ScalarE hop on the recurrence critical path.  The activation is
    split per 128-dim chunk so each chunk unblocks the next layer's
    matmuls as soon as its PSUM accumulation group completes.
  - The head + log-softmax + token-gather do NOT feed the recurrence, so
    they are deferred: the 128 "read" dims per step are stored to a
    [128, T*8] buffer and processed as 32 dense batched matmul tiles
    after the T-loop.  No max-subtraction needed: |logits| <= 8.
  - Each core returns per-partition partial sums of (logsumexp - logit_tok);
    the host sums across cores and divides by B*T.
"""

import math
import sys

import numpy as np

sys.path.insert(0, "/opt/trn_rl_repo")

import ml_dtypes  # noqa: E402

import concourse.bass as bass  # noqa: E402
import concourse.bacc as bacc  # noqa: E402
import concourse.mybir as mybir  # noqa: E402
from concourse.tile import TileContext  # noqa: E402
from concourse.bass_utils import run_bass_kernel_spmd  # noqa: E402

F32 = mybir.dt.float32
BF16 = mybir.dt.bfloat16
FP8 = mybir.dt.float8e4
NP_BF16 = ml_dtypes.bfloat16
NP_FP8 = ml_dtypes.float8_e4m3

D = 1024          # d_model
KC = 8            # contraction chunks of 128
MC = 8            # output chunks of 128
NUMFF = 6
VOCAB = 128
READ = 128
CARRY = 896
BL = 8            # batch per core
NCORES = 8
LOGIT_SCALE = 1.0 / 16.0

# weight dtype for the FF stack / head (fp8 -> fast weight load)
W_DT = FP8
W_NP = NP_FP8


def build_nc(T):
    """Build the SPMD Bass kernel for T timesteps (BL sequences/core)."""
    ntile = T * BL // 128  # tail tiles over (t, b)
    assert T * BL % 128 == 0

    nc = bacc.Bacc("TRN2", target_bir_lowering=False)
    wff = nc.dram_tensor("wff", [128, NUMFF * KC * D], W_DT, kind="ExternalInput")
    emb = nc.dram_tensor("emb", [128, T * BL], BF16, kind="ExternalInput")
    x0 = nc.dram_tensor("x0", [128, MC * BL], BF16, kind="ExternalInput")
    thr = nc.dram_tensor("thr", [128, NUMFF * MC * BL], F32, kind="ExternalInput")
    headw = nc.dram_tensor("headw", [128, VOCAB], W_DT, kind="ExternalInput")
    oneh = nc.dram_tensor("oneh", [128, ntile * VOCAB], F32, kind="ExternalInput")
    wexp = nc.dram_tensor("wexp", [128, VOCAB], F32, kind="ExternalInput")
    res = nc.dram_tensor("res", [128, 1], F32, kind="ExternalOutput")

    AT = mybir.ActivationFunctionType
    ALU = mybir.AluOpType

    with TileContext(nc) as tc:
        with (
            tc.tile_pool(name="const", bufs=1) as cpool,
            tc.tile_pool(name="work", bufs=4) as wpool,
            tc.tile_pool(name="hpool", bufs=3) as hpool,
            tc.tile_pool(name="pst", bufs=1, space="PSUM") as pstpool,
            tc.tile_pool(name="ps2pool", bufs=2, space="PSUM") as ps2pool,
        ):
            # ---- resident inputs ----
            wsb = cpool.tile([128, NUMFF * KC * D], W_DT, tag="wsb")
            for i in range(NUMFF):
                nc.sync.dma_start(
                    out=wsb[:, i * KC * D : (i + 1) * KC * D],
                    in_=wff[:, i * KC * D : (i + 1) * KC * D],
                )
            embsb = cpool.tile([128, T * BL], BF16, tag="embsb")
            nc.sync.dma_start(out=embsb[:, :], in_=emb[:, :])
            thrsb = cpool.tile([128, NUMFF * MC * BL], F32, tag="thrsb")
            nc.sync.dma_start(out=thrsb[:, :], in_=thr[:, :])
            headsb = cpool.tile([128, VOCAB], W_DT, tag="headsb")
            nc.sync.dma_start(out=headsb[:, :], in_=headw[:, :])
            onehsb = cpool.tile([128, ntile * VOCAB], F32, tag="onehsb")
            nc.sync.dma_start(out=onehsb[:, :], in_=oneh[:, :])
            wexpsb = cpool.tile([128, VOCAB], F32, tag="wexpsb")
            nc.sync.dma_start(out=wexpsb[:, :], in_=wexp[:, :])
            xb = cpool.tile([128, MC * BL], BF16, tag="xb")
            nc.sync.dma_start(out=xb[:, :], in_=x0[:, :])

            # ---- persistent state ----
            readst = cpool.tile([128, T * BL], BF16, tag="readst")
            sumexp = cpool.tile([128, ntile], F32, tag="sumexp")
            tokl = cpool.tile([128, ntile], F32, tag="tokl")
            # 4 cycling PSUM tiles (one bank each) so DVE activation reads
            # never share a bank with in-flight PE writes (reuse distance
            # = 4 accumulation groups)
            NPS = 4
            psts = [
                pstpool.tile([128, MC * BL // NPS], F32, name=f"pst{j}", tag=f"pst{j}")
                for j in range(NPS)
            ]

            # ---- recurrence ----
            for t in range(T):
                src = xb
                for i in range(NUMFF):
                    last = i == NUMFF - 1
                    h = None if last else hpool.tile([128, MC * BL], BF16, tag="h")
                    for m in range(MC):
                        ps = psts[m % NPS]
                        pc = (m // NPS) * BL
                        lo = (i * KC) * D + m * 128
                        for k in range(KC):
                            nc.tensor.matmul(
                                ps[:, pc : pc + BL],
                                wsb[:, k * D + lo : k * D + lo + 128],
                                src[:, k * BL : (k + 1) * BL],
                                start=(k == 0),
                                stop=(k == KC - 1),
                            )
                        # per-chunk activation: h'[m] = (pre' >= thr2) in {1,0}
                        if last:
                            dst = (
                                readst[:, t * BL : (t + 1) * BL]
                                if m == MC - 1
                                else xb[:, m * BL : (m + 1) * BL]
                            )
                        else:
                            dst = h[:, m * BL : (m + 1) * BL]
                        nc.vector.scalar_tensor_tensor(
                            out=dst,
                            in0=ps[:, pc : pc + BL],
                            scalar=0.0,
                            in1=thrsb[:, i * MC * BL + m * BL : i * MC * BL + (m + 1) * BL],
                            op0=ALU.add,
                            op1=ALU.is_ge,
                        )
                    if i == 0:
                        # refill the embed chunk of x for the NEXT step as
                        # soon as this step's layer-0 matmuls consumed it
                        nc.vector.tensor_copy(
                            xb[:, CARRY // 16 : MC * BL],
                            embsb[:, t * BL : (t + 1) * BL],
                        )
                    if not last:
                        src = h

            # ---- deferred head + loss ----
            for j in range(ntile):
                ps2 = ps2pool.tile([128, VOCAB], F32, tag="ps2")
                nc.tensor.matmul(
                    ps2[:, :],
                    readst[:, j * 128 : (j + 1) * 128],
                    headsb[:, :],
                    start=True,
                    stop=True,
                )
                # logits = (2*ps2 - colsum(H)) / 16 ; exp(logits) =
                # exp(ps2/8) * wexp  with wexp = exp(-colsum(H)/16)
                expt = wpool.tile([128, VOCAB], F32, tag="expt")
                nc.scalar.activation(
                    expt[:, :],
                    ps2[:, :],
                    AT.Exp,
                    scale=2.0 * LOGIT_SCALE,
                )
                junk2 = wpool.tile([128, VOCAB], F32, tag="junk2")
                nc.vector.scalar_tensor_tensor(
                    out=junk2[:, :],
                    in0=expt[:, :],
                    scalar=1.0,
                    in1=wexpsb[:, :],
                    op0=ALU.mult,
                    op1=ALU.mult,
                    accum_out=sumexp[:, j : j + 1],
                )
                # device part of logit_tok: (2/16) * ps2_tok (host adds the
                # -colsum(H)[tok]/16 correction)
                junk = wpool.tile([128, VOCAB], F32, tag="junk")
                nc.vector.scalar_tensor_tensor(
                    out=junk[:, :],
                    in0=ps2[:, :],
                    scalar=2.0 * LOGIT_SCALE,
                    in1=onehsb[:, j * VOCAB : (j + 1) * VOCAB],
                    op0=ALU.mult,
                    op1=ALU.mult,
                    accum_out=tokl[:, j : j + 1],
                )

            lse = cpool.tile([128, ntile], F32, tag="lse")
            nc.scalar.activation(lse[:, :], sumexp[:, :], AT.Ln)
            r1 = cpool.tile([128, 1], F32, tag="r1")
            r2 = cpool.tile([128, 1], F32, tag="r2")
            po = cpool.tile([128, 1], F32, tag="po")
            nc.vector.tensor_reduce(
                r1[:, :], lse[:, :], axis=mybir.AxisListType.X, op=ALU.add
            )
            nc.vector.tensor_reduce(
                r2[:, :], tokl[:, :], axis=mybir.AxisListType.X, op=ALU.add
            )
            nc.vector.scalar_tensor_tensor(
                out=po[:, :],
                in0=r1[:, :],
                scalar=0.0,
                in1=r2[:, :],
                op0=ALU.add,
                op1=ALU.subtract,
            )
            nc.sync.dma_start(out=res[:, :], in_=po[:, :])

    return nc


def _sgn(a):
    return np.where(np.asarray(a, np.float32) >= 0, 1.0, -1.0).astype(np.float32)


def prepare_inputs(tokens, initial_lat, embed_lat, ff_lat, head_lat, ff_thresh_lat):
    """Host-side transform of the full problem inputs to per-core DRAM maps.

    Activations are sent in {0,1} encoding h' = (h+1)/2, with thresholds
    folded:  pre >= thr  <=>  h'@W >= (thr + colsum(W))/2.
    Returns (in_maps, B, T, host_corr) where host_corr is the token-logit
    correction  sum_{b,t} colsum(H)[tok]/16  to add to the loss sum.
    """
    tokens = np.asarray(tokens).astype(np.int64)
    B, T = tokens.shape
    E = _sgn(embed_lat)                      # [V, 128]
    W = _sgn(ff_lat)                         # [6, 1024, 1024]
    H = _sgn(head_lat)                       # [128, V]
    x0v = _sgn(initial_lat)                  # [1024]
    R = np.round(np.asarray(ff_thresh_lat, np.float64)).astype(np.float32)  # [6,1024]

    # weights: wff[p, (i*KC+k)*D + mcol] = W[i, k*128+p, mcol]
    wff = (
        W.reshape(NUMFF, KC, 128, D).transpose(2, 0, 1, 3).reshape(128, NUMFF * KC * D)
    ).astype(W_NP)
    headw = H.astype(W_NP)                   # [128 r, V]

    # x0 in {0,1}: col = chunk*8 + b, value (x0v+1)/2 replicated over b
    x0t = ((x0v + 1.0) / 2.0).reshape(MC, 128).T     # [p, chunk]
    x0arr = np.repeat(x0t[:, :, None], BL, axis=2).reshape(128, MC * BL).astype(NP_BF16)

    # folded threshold thr2 = (thr + colsum(W))/2, expanded col = i*64+m*8+b
    S = W.sum(axis=1)                        # [6, 1024] colsums
    thr2 = (R + S) / 2.0
    thr2 = thr2.reshape(NUMFF, MC, 128).transpose(2, 0, 1)[:, :, :, None]
    thr2 = np.broadcast_to(thr2, (128, NUMFF, MC, BL)).reshape(128, NUMFF * MC * BL)
    thr2 = np.ascontiguousarray(thr2, np.float32)

    # head colsum corrections
    csH = H.sum(axis=0)                      # [V]
    wexp = np.exp(-csH / 16.0).astype(np.float32)
    wexp = np.ascontiguousarray(np.broadcast_to(wexp[None, :], (128, VOCAB)))
    host_corr = float(csH[tokens].sum()) / 16.0

    ntile = T * BL // 128
    in_maps = []
    for c in range(NCORES):
        tc_ = tokens[c * BL : (c + 1) * BL]  # [8, T]
        # emb in {0,1}: emb[p, t*8+b] = (E[tok[b,t], p]+1)/2
        embc = ((E[tc_] + 1.0) / 2.0).transpose(2, 1, 0).reshape(128, T * BL)
        embc = embc.astype(NP_BF16)
        # one-hot over vocab per (t,b) sample, tiled [128 samples x 128 vocab]
        flat = tc_.T.reshape(-1)             # tb = t*8+b
        onehc = (flat[:, None] == np.arange(VOCAB)[None, :]).astype(np.float32)
        onehc = (
            onehc.reshape(ntile, 128, VOCAB).transpose(1, 0, 2).reshape(128, ntile * VOCAB)
        )
        onehc = np.ascontiguousarray(onehc)
        in_maps.append(
            {
                "wff": wff,
                "emb": embc,
                "x0": x0arr,
                "thr": thr2,
                "headw": headw,
                "oneh": onehc,
                "wexp": wexp,
            }
        )
    return in_maps, B, T, host_corr


def _install_axon_trace_hook():
    """The image's antenv lacks axon_hooks; recreate the NTFF profile hook
    via ctypes against libaxon_pjrt.so (mirrors trn_agent_boot.trn_boot)."""
    import contextlib
    import ctypes
    import types

    try:
        from antenv.axon_hooks import get_axon_ntff_profile_hook  # noqa: F401

        return
    except ImportError:
        pass
    so_path = "/opt/axon/libaxon_pjrt.so"
    lib = ctypes.CDLL(so_path)
    if not hasattr(lib, "axon_start_nrt_profile"):
        return
    lib.axon_start_nrt_profile.argtypes = [
        ctypes.POINTER(ctypes.c_int64),
        ctypes.c_size_t,
    ]
    lib.axon_start_nrt_profile.restype = ctypes.c_int64
    lib.axon_stop_nrt_profile.argtypes = [ctypes.c_char_p]
    lib.axon_stop_nrt_profile.restype = ctypes.c_int64

    @contextlib.contextmanager
    def _hook(output_dir, device_ids):
        import jax

        jax.devices()
        if device_ids:
            ids = (ctypes.c_int64 * len(device_ids))(*device_ids)
            rc = lib.axon_start_nrt_profile(ids, len(device_ids))
        else:
            rc = lib.axon_start_nrt_profile(None, 0)
        if rc != 0:
            raise RuntimeError(f"axon_start_nrt_profile rc={rc}")
        try:
            yield
        finally:
            n = lib.axon_stop_nrt_profile(str(output_dir).encode())
            print(f"profile: {n} file(s) written to {output_dir}", file=sys.stderr)

    import antenv

    mod = types.ModuleType("antenv.axon_hooks")
    mod.get_axon_ntff_profile_hook = lambda: _hook
    sys.modules["antenv.axon_hooks"] = mod
    antenv.axon_hooks = mod

    from concourse import bass_utils as bu

    bu.upload_artifacts = lambda tmpdir: f"local://{tmpdir}"


def run(trace=False, tmpdir=None, **inputs):
    in_maps, B, T, host_corr = prepare_inputs(**inputs)
    nc = build_nc(T)
    if not nc.is_finalized():
        nc.finalize()
    if trace:
        _install_axon_trace_hook()
    out = run_bass_kernel_spmd(
        nc, in_maps, core_ids=list(range(NCORES)), trace=trace, tmpdir=tmpdir
    )
    total = host_corr
    for r in out.results:
        total += np.asarray(r["res"], np.float64).sum()
    loss = np.float32(total / (B * T))
    return np.asarray(loss, dtype=np.float32), out


def kernel(**inputs):
    loss, _ = run(trace=False, **inputs)
    return loss


if __name__ == "__main__":
    # tiny smoke test
    import jax

    sys.path.insert(0, "/root/problem")
    import reference

    inputs = reference.setup_inputs()
    inputs = {k: np.asarray(v) for k, v in inputs.items()}
    Tsmall = int(sys.argv[1]) if len(sys.argv) > 1 else 16
    inputs["tokens"] = inputs["tokens"][:, :Tsmall]
    expected = np.asarray(reference.reference(**{k: v for k, v in inputs.items()}))
    got = kernel(**inputs)
    rel = abs(float(got) - float(expected)) / max(1e-12, abs(float(expected)))
    print(f"T={Tsmall} expected={expected} got={got} rel_err={rel:.3e}")
